# revision 29
# baseline (speedup 1.0000x reference)
"""GCN VGAE encoder (2-layer, mu/logstd heads) on 8 Trainium2 NeuronCores.

Strategy (edge-parallel over dst-sorted runs, bf16 datapath):
  - Host: per core (dst shard), sort edges by (dst block j, q = src_lane%4).
    Nodes are packed 4-per-256B-row in the gather tables (row index fits
    int16: 25088 rows total), so ONE table serves all cores' sources (no
    quarter split) and the only padding is 32-lane alignment of q segments
    plus 128-lane chunk alignment, maxed across cores (SPMD-uniform).
  - Device (SPMD x8, each core owns 98 node blocks = 12544 nodes):
      disv = 1/sqrt(deg+1); y1 = disv*x cast bf16, written (16B/row) into a
      pack-4 table.  L1: dma_gather 256B rows per 128-edge chunk (4 SWDGE
      queues), one-hot via DVE is_equal (bf16); per-piece matmuls (32-lane
      aligned) select the q sub-block of each gathered row; self-loop via
      ident-rhs matmul; h = relu(disv*(s1@W1)+b1);
      z = disv*(h@[Wmu|Wls]) (heads pre-applied, 32 wide) written to a
      DENSE pack-4 table (256B/row, no waste).
      AllGather the compact z shards (802KB/core in, 6.4MB table out).
      L2: same gather structure (same int16 indices), flipped matmuls
      (one-hot stationary, FWL) accumulate [128 dst, 32] in PSUM;
      self-loop via ident lhsT; out = disv*psum + bias, split mu/logstd.
All floating-point math runs on device; the host only reorders integers.
"""
import numpy as np

import concourse.bass as bass
import concourse.bacc as bacc
import concourse.mybir as mybir
import concourse.tile as tile
from concourse.bass_utils import run_bass_kernel_spmd
from concourse.masks import make_identity

P = 128
N_CORES = 8
NQ = 4                      # SWDGE queues (hw max) == pack factor
F32 = mybir.dt.float32
BF16 = mybir.dt.bfloat16
I32 = mybir.dt.int32
I16 = mybir.dt.int16

_CACHE = {}


def _ceil(a, b):
    return -(-a // b)


# ---------------------------------------------------------------- host prep
def _prep(x, edge_index):
    N = x.shape[0]
    in_ch = x.shape[1]
    nbc = _ceil(_ceil(N, N_CORES), P)                # blocks per core (98)
    npc = nbc * P                                    # nodes per core (12544)
    npad = N_CORES * npc                             # padded nodes (100352)
    nblk = N_CORES * nbc                             # blocks (784)
    rpc = (P // 4) * nbc                             # table rows per core (3136)
    nrow = N_CORES * rpc                             # table rows (25088)

    src = np.asarray(edge_index[0]).astype(np.int64)
    dst = np.asarray(edge_index[1]).astype(np.int64)
    E = src.shape[0]

    deg = np.bincount(dst, minlength=npad).astype(np.int32)

    # per-core block permutation (sort blocks by edge count) aligns big
    # blocks across cores, shrinking the max-over-cores padding
    cnt_blk = np.bincount(dst >> 7, minlength=nblk).reshape(N_CORES, nbc)
    perm = np.argsort(-cnt_blk, axis=1)                        # [NC, nbc]
    pos = np.empty_like(perm)
    for c in range(N_CORES):
        pos[c, perm[c]] = np.arange(nbc)
    posg = pos.reshape(-1)

    # edge fields
    c_d = dst // npc
    j_e = posg[dst >> 7]                             # dst block position
    c_s = src // npc
    p_s = src & 127                                  # src lane
    pos_s = posg[src >> 7]                           # src block position
    # both tables pack lanes 4g..4g+3 of one block per 256B row, but row
    # ORDER differs: y1 table is g-major (clean 3-dim DMA write), z table is
    # core-major (AllGather concatenation order)
    row1_e = (p_s >> 2) * nblk + c_s * nbc + pos_s   # y1 table row
    row2_e = c_s * rpc + (p_s >> 2) * nbc + pos_s    # z table row
    q_e = p_s & 3                                    # sub-block within row

    # sort per (core, j, q)
    order = np.argsort((c_d * nbc + j_e) * 4 + q_e, kind="stable")
    run_id = ((c_d * nbc + j_e) * 4 + q_e)[order]
    nrun = nblk * 4
    counts = np.bincount(run_id, minlength=nrun)
    rstart = np.zeros(nrun + 1, np.int64)
    np.cumsum(counts, out=rstart[1:])

    # segment sizes maxed over cores (SPMD-uniform); no lane alignment
    # needed -- every matmul runs all 128 lanes and masking lives in the
    # per-piece one-hot column (lanes outside the piece get dstl=128)
    n_cjq = counts.reshape(N_CORES, nbc, 4)
    m_jq = n_cjq.max(axis=0).astype(np.int64)                  # [nbc, 4]
    L_j = 128 * _ceil(m_jq.sum(axis=1), 128)                   # lanes per j
    qoff = np.zeros((nbc, 5), np.int64)
    np.cumsum(m_jq, axis=1, out=qoff[:, 1:])
    T_j = L_j // 128
    off = np.zeros(nbc + 1, np.int64)
    np.cumsum(T_j, out=off[1:])
    T = int(off[-1])

    # pieces per j: (t, q, a, b) = chunk t, sub-block q, lanes [a, b)
    pieces = []
    for j in range(nbc):
        pj = []
        for t in range(int(T_j[j])):
            lo, hi = 128 * t, 128 * (t + 1)
            for q in range(4):
                s0, s1 = int(qoff[j, q]), int(qoff[j, q + 1])
                a, b = max(lo, s0), min(hi, s1)
                if a < b:
                    pj.append((t, q, a - lo, b - lo))
        pieces.append(tuple(pj))
    np_j = tuple(len(pj) for pj in pieces)
    npoff = np.zeros(nbc + 1, np.int64)
    np.cumsum(np_j, out=npoff[1:])
    NP = int(npoff[-1])

    dst_s = dst[order]
    row1_s = row1_e[order]
    row2_s = row2_e[order]

    dstc = np.full((N_CORES, P, T), 128.0, np.float32)   # per-chunk dst lane
    idx1 = np.zeros((N_CORES, P, T), np.int16)
    idx2 = np.zeros((N_CORES, P, T), np.int16)
    for c in range(N_CORES):
        for j in range(nbc):
            for q in range(4):
                r = (c * nbc + j) * 4 + q
                e0, e1 = int(rstart[r]), int(rstart[r + 1])
                n_e = e1 - e0
                if n_e == 0:
                    continue
                i = int(qoff[j, q]) + np.arange(n_e)
                lane = i & 127
                col = int(off[j]) + (i >> 7)
                dstc[c, lane, col] = (dst_s[e0:e1] & 127).astype(np.float32)
                idx1[c, lane, col] = row1_s[e0:e1].astype(np.int16)
                idx2[c, lane, col] = row2_s[e0:e1].astype(np.int16)

    # per-piece masked dst-lane columns (one one-hot column per piece)
    dstl = np.full((N_CORES, P, NP), 128.0, np.float32)
    lanes = np.arange(P)
    for j in range(nbc):
        for i, (t, q, a, b) in enumerate(pieces[j]):
            pcol = int(npoff[j]) + i
            col = int(off[j]) + t
            m = (lanes >= a) & (lanes < b)
            dstl[:, m, pcol] = dstc[:, m, col]

    # queue assignment: chunk cidx -> queue cidx%NQ; wrapped int16 layout
    nq_chunks = tuple(len(range(q, T, NQ)) for q in range(NQ))

    def wrap(idx16):
        per_q = []
        for q in range(NQ):
            cols = np.arange(q, T, NQ)
            per_c = []
            for c in range(N_CORES):
                flat = idx16[c][:, cols].T.ravel()    # chunk-major lane-minor
                w16 = flat.reshape(-1, 16).T          # [16, nq*8]
                per_c.append(np.tile(w16, (8, 1)))    # [128, nq*8]
            per_q.append(np.stack(per_c))
        return per_q

    idxw1, idxw2 = wrap(idx1), wrap(idx2)

    # replicated node arrays in permuted block order (lane, core-major block)
    gidx = np.concatenate([c * nbc + perm[c] for c in range(N_CORES)])
    xpad = np.zeros((npad, in_ch), np.float32)
    xpad[:N] = np.asarray(x, np.float32)
    x_g = xpad.reshape(nblk, P, in_ch)[gidx].transpose(1, 0, 2).reshape(P, -1)
    deg_g = deg.reshape(nblk, P)[gidx].T.copy()
    x_own = np.stack([
        x_g[:, c * nbc * in_ch:(c + 1) * nbc * in_ch] for c in range(N_CORES)])
    deg_own = np.stack([deg_g[:, c * nbc:(c + 1) * nbc] for c in range(N_CORES)])

    iota = np.tile(np.arange(P, dtype=np.float32), (P, 1))

    meta = dict(N=N, E=E, in_ch=in_ch, nbc=nbc, npc=npc, npad=npad,
                nblk=nblk, rpc=rpc, nrow=nrow, T=T, NP=NP, perm=perm,
                off=off, npoff=npoff, T_j=tuple(int(t) for t in T_j),
                nq_chunks=nq_chunks, pieces=tuple(pieces))
    arrays = dict(dstl=dstl, idxw1=idxw1, idxw2=idxw2, x_g=x_g, deg_g=deg_g,
                  x_own=x_own, deg_own=deg_own, iota=iota)
    return meta, arrays


# ---------------------------------------------------------------- device build
def _build(meta, in_ch, hid, zw):
    nbc, nblk, T = meta["nbc"], meta["nblk"], meta["T"]
    rpc, nrow, NP = meta["rpc"], meta["nrow"], meta["NP"]
    off, npoff, pieces = meta["off"], meta["npoff"], meta["pieces"]
    nq_chunks = meta["nq_chunks"]
    ow = zw // 2             # per-head output width (16)
    SPC = 14                 # chunks per gather instruction

    nc = bacc.Bacc("TRN2", target_bir_lowering=False, debug=False,
                   num_devices=N_CORES, num_swdge_queues=NQ)

    x_g_d = nc.dram_tensor("x_g", [P, nblk * in_ch], F32, kind="ExternalInput")
    x_o_d = nc.dram_tensor("x_own", [P, nbc * in_ch], F32, kind="ExternalInput")
    deg_g_d = nc.dram_tensor("deg_g", [P, nblk], I32, kind="ExternalInput")
    deg_o_d = nc.dram_tensor("deg_own", [P, nbc], I32, kind="ExternalInput")
    dstl_d = nc.dram_tensor("dstl", [P, NP], F32, kind="ExternalInput")
    idxw1_d = [nc.dram_tensor(f"idxw1_{q}", [P, nq_chunks[q] * 8], I16,
                              kind="ExternalInput") for q in range(NQ)]
    idxw2_d = [nc.dram_tensor(f"idxw2_{q}", [P, nq_chunks[q] * 8], I16,
                              kind="ExternalInput") for q in range(NQ)]
    iota_d = nc.dram_tensor("iota", [P, P], F32, kind="ExternalInput")
    w1_d = nc.dram_tensor("w1", [in_ch, hid], F32, kind="ExternalInput")
    b1_d = nc.dram_tensor("b1", [P, hid], F32, kind="ExternalInput")
    wz_d = nc.dram_tensor("wz", [hid, zw], F32, kind="ExternalInput")
    bz_d = nc.dram_tensor("bz", [P, zw], F32, kind="ExternalInput")
    mu_o = nc.dram_tensor("mu_o", [P, nbc * ow], F32, kind="ExternalOutput")
    ls_o = nc.dram_tensor("ls_o", [P, nbc * ow], F32, kind="ExternalOutput")

    y1tab = nc.dram_tensor("y1tab", [nrow, P], BF16, kind="Internal")
    cc_in = nc.dram_tensor("cc_in", [rpc, P], BF16, kind="Internal")
    cc_out = nc.dram_tensor("cc_out", [nrow, P], BF16,
                            kind="Internal", addr_space="Shared")

    spans = []               # per queue: list of (chunk0, n)
    for q in range(NQ):
        sp = []
        for c0 in range(0, nq_chunks[q], SPC):
            sp.append((c0, min(SPC, nq_chunks[q] - c0)))
        spans.append(sp)
    max_spans = max(len(s) for s in spans)
    order_g = [(q, spans[q][i]) for i in range(max_spans)
               for q in range(NQ) if i < len(spans[q])]

    def emit_gathers(pool, table_ap, idxw_t, tag):
        gtiles = {q: [] for q in range(NQ)}
        for q, (c0, n) in order_g:
            gt = pool.tile([P, n * P], BF16, tag=tag)
            nc.gpsimd.dma_gather(
                out_ap=gt[:].rearrange("p (n e) -> p n e", e=P),
                in_ap=table_ap,
                idxs_ap=idxw_t[q][:, c0 * 8:(c0 + n) * 8],
                num_idxs=n * P, num_idxs_reg=n * P,
                elem_size=P, single_packet=False, queue_num=q)
            gtiles[q].append((c0, n, gt))
        return gtiles

    def msg(gtiles, cidx):
        q, qpos = cidx % NQ, cidx // NQ
        s, k = qpos // SPC, qpos % SPC
        c0, n, gt = gtiles[q][s]
        assert c0 <= qpos < c0 + n
        return gt, qpos - c0

    with tile.TileContext(nc) as tc:
        with tc.tile_pool(name="const", bufs=1) as cp:
            dstl_t = cp.tile([P, NP], BF16)
            dstl_f = cp.tile([P, NP], F32)
            idxw1_t = [cp.tile([P, nq_chunks[q] * 8], I16, name=f"idxw1{q}_t")
                       for q in range(NQ)]
            idxw2_t = [cp.tile([P, nq_chunks[q] * 8], I16, name=f"idxw2{q}_t")
                       for q in range(NQ)]
            iota_t = cp.tile([P, P], BF16)
            iota_f = cp.tile([P, P], F32)
            x_gt = cp.tile([P, nblk * in_ch], F32)
            x_ot = cp.tile([P, nbc * in_ch], F32)
            deg_gt = cp.tile([P, nblk], I32)
            deg_ot = cp.tile([P, nbc], I32)
            w1_f = cp.tile([in_ch, hid], F32)
            w1_t = cp.tile([in_ch, hid], BF16)
            b1_t = cp.tile([P, hid], F32)
            wz_f = cp.tile([hid, zw], F32)
            wz_t = cp.tile([hid, zw], BF16)
            bz_t = cp.tile([P, zw], F32)
            ident = cp.tile([P, P], BF16)
            disv_g = cp.tile([P, nblk], F32)
            disv_o = cp.tile([P, nbc], F32)
            y1_bf = cp.tile([P, nblk * in_ch], BF16)
            y1_own = cp.tile([P, nbc * in_ch], BF16)
            hpre = cp.tile([P, nbc * hid], F32)
            h_bf = cp.tile([P, nbc * hid], BF16)
            z_sb = cp.tile([P, nbc * zw], BF16)
            msb = cp.tile([P, nbc * zw], F32)

            for dt_, tt in ((x_g_d, x_gt), (x_o_d, x_ot), (deg_g_d, deg_gt),
                            (deg_o_d, deg_ot), (dstl_d, dstl_f),
                            (iota_d, iota_f), (w1_d, w1_f), (b1_d, b1_t),
                            (wz_d, wz_f), (bz_d, bz_t)):
                nc.sync.dma_start(out=tt[:], in_=dt_[:])
            for q in range(NQ):
                nc.sync.dma_start(out=idxw1_t[q][:], in_=idxw1_d[q][:])
                nc.sync.dma_start(out=idxw2_t[q][:], in_=idxw2_d[q][:])
            make_identity(nc, ident[:])
            nc.vector.tensor_copy(out=dstl_t[:], in_=dstl_f[:])
            nc.vector.tensor_copy(out=iota_t[:], in_=iota_f[:])
            nc.vector.tensor_copy(out=w1_t[:], in_=w1_f[:])
            nc.vector.tensor_copy(out=wz_t[:], in_=wz_f[:])

            # disv = 1/sqrt(deg+1) (global + own)
            for deg_t, dv in ((deg_gt, disv_g), (deg_ot, disv_o)):
                nc.vector.tensor_copy(out=dv[:], in_=deg_t[:])
                nc.scalar.activation(dv[:], dv[:],
                                     mybir.ActivationFunctionType.Sqrt,
                                     bias=1.0)
                nc.vector.reciprocal(out=dv[:], in_=dv[:])

            # y1 = disv * x (bf16), replicated + own
            nc.vector.tensor_tensor(
                out=y1_bf[:].rearrange("p (b c) -> p b c", c=in_ch),
                in0=x_gt[:].rearrange("p (b c) -> p b c", c=in_ch),
                in1=disv_g[:].rearrange("p (b o) -> p b o", o=1)
                    .to_broadcast([P, nblk, in_ch]),
                op=mybir.AluOpType.mult)
            nc.vector.tensor_tensor(
                out=y1_own[:].rearrange("p (b c) -> p b c", c=in_ch),
                in0=x_ot[:].rearrange("p (b c) -> p b c", c=in_ch),
                in1=disv_o[:].rearrange("p (b o) -> p b o", o=1)
                    .to_broadcast([P, nbc, in_ch]),
                op=mybir.AluOpType.mult)

            # write pack-4 y1 table (8 useful bf16 per 256B row); rows are
            # g-major so each qp is one clean 3-dim DMA
            vt = y1tab.ap().rearrange("(g cb) w -> g cb w", g=P // 4)
            for qp in range(4):
                nc.scalar.dma_start(
                    out=vt[:, :, qp * in_ch:(qp + 1) * in_ch],
                    in_=y1_bf[:].rearrange("(g qp) x -> g qp x",
                                           qp=4)[:, qp])

            # ---------------- Layer 1 ----------------
            with (
                tc.tile_pool(name="g1", bufs=12) as g1p,
                tc.tile_pool(name="oh1", bufs=3) as ohp,
                tc.tile_pool(name="s1", bufs=4) as s1p,
                tc.tile_pool(name="ps1", bufs=3, space="PSUM") as pp1,
                tc.tile_pool(name="psh", bufs=3, space="PSUM") as pph,
            ):
                gtiles1 = emit_gathers(g1p, y1tab.ap()[:, :], idxw1_t, "g1")

                for j in range(nbc):
                    nj = int(npoff[j + 1] - npoff[j])
                    p0 = int(npoff[j])
                    c0 = int(off[j])
                    oh = ohp.tile([P, nj * P], BF16, tag="oh")
                    nc.vector.tensor_tensor(
                        out=oh[:].rearrange("p (s f) -> p s f", f=P),
                        in0=iota_t[:].rearrange("p (o f) -> p o f", o=1)
                            .to_broadcast([P, nj, P]),
                        in1=dstl_t[:, p0:p0 + nj]
                            .rearrange("p (s o) -> p s o", o=1)
                            .to_broadcast([P, nj, P]),
                        op=mybir.AluOpType.is_equal)
                    ps1 = pp1.tile([in_ch, P], F32, tag="ps1")
                    first = True
                    for i, (t, q, a, b) in enumerate(pieces[j]):
                        gt, k = msg(gtiles1, c0 + t)
                        nc.tensor.matmul(
                            out=ps1[:],
                            lhsT=gt[:, k * P + q * in_ch:
                                    k * P + (q + 1) * in_ch],
                            rhs=oh[:, i * P:(i + 1) * P],
                            start=first, stop=False)
                        first = False
                    # self-loop: += y1_own[:, j]^T via ident rhs
                    nc.tensor.matmul(
                        out=ps1[:],
                        lhsT=y1_own[:, j * in_ch:(j + 1) * in_ch],
                        rhs=ident[:], start=first, stop=True)
                    s1T = s1p.tile([in_ch, P], BF16, tag="s1T")
                    nc.vector.tensor_copy(out=s1T[:], in_=ps1[:])
                    ph = pph.tile([P, hid], F32, tag="ph")
                    nc.tensor.matmul(out=ph[:], lhsT=s1T[:], rhs=w1_t[:],
                                     start=True, stop=True)
                    nc.vector.tensor_tensor(
                        out=hpre[:, j * hid:(j + 1) * hid],
                        in0=ph[:],
                        in1=disv_o[:, j:j + 1].to_broadcast([P, hid]),
                        op=mybir.AluOpType.mult)

            # h = relu(hpre + b1) cast bf16
            nc.vector.tensor_tensor(
                out=hpre[:].rearrange("p (b c) -> p b c", c=hid),
                in0=hpre[:].rearrange("p (b c) -> p b c", c=hid),
                in1=b1_t[:].rearrange("p (o c) -> p o c", o=1)
                    .to_broadcast([P, nbc, hid]),
                op=mybir.AluOpType.add)
            nc.scalar.activation(h_bf[:], hpre[:],
                                 mybir.ActivationFunctionType.Relu)

            # z = disv * (h @ Wz) per block (transpose h, matmul, scale)
            with (
                tc.tile_pool(name="hTb", bufs=4) as hTp,
                tc.tile_pool(name="psz", bufs=3, space="PSUM") as ppz,
                tc.tile_pool(name="psz2", bufs=3, space="PSUM") as ppz2,
            ):
                for j in range(nbc):
                    hT = ppz.tile([hid, P], F32, tag="hT")
                    nc.tensor.matmul(
                        out=hT[:], lhsT=h_bf[:, j * hid:(j + 1) * hid],
                        rhs=ident[:], start=True, stop=True)
                    hTb = hTp.tile([hid, P], BF16, tag="hTb")
                    nc.vector.tensor_copy(out=hTb[:], in_=hT[:])
                    zp = ppz2.tile([P, zw], F32, tag="zp")
                    nc.tensor.matmul(out=zp[:], lhsT=hTb[:], rhs=wz_t[:],
                                     start=True, stop=True)
                    nc.vector.tensor_tensor(
                        out=z_sb[:, j * zw:(j + 1) * zw],
                        in0=zp[:],
                        in1=disv_o[:, j:j + 1].to_broadcast([P, zw]),
                        op=mybir.AluOpType.mult)

            # cc_in: pack-4 dense z rows; one DMA per sub-slot qp
            vc = cc_in.ap().rearrange("(g pos) (qp w) -> g pos qp w",
                                      g=P // 4, qp=4)
            for qp in range(4):
                nc.scalar.dma_start(
                    out=vc[:, :, qp],
                    in_=z_sb[:].rearrange("(g qp) (pos w) -> g qp pos w",
                                          qp=4, w=zw)[:, qp])
            nc.gpsimd.collective_compute(
                "AllGather", mybir.AluOpType.bypass,
                ins=[cc_in[:]], outs=[cc_out[:]],
                replica_groups=[list(range(N_CORES))])

            # ---------------- Layer 2 ----------------
            with (
                tc.tile_pool(name="g2", bufs=12) as g2p,
                tc.tile_pool(name="oh2", bufs=3) as ohp2,
                tc.tile_pool(name="ps2", bufs=4, space="PSUM") as pp2,
            ):
                gtiles2 = emit_gathers(g2p, cc_out.ap()[:, :], idxw2_t, "g2")

                for j in range(nbc):
                    nj = int(npoff[j + 1] - npoff[j])
                    p0 = int(npoff[j])
                    c0 = int(off[j])
                    oh = ohp2.tile([P, nj * P], BF16, tag="oh2")
                    nc.vector.tensor_tensor(
                        out=oh[:].rearrange("p (s f) -> p s f", f=P),
                        in0=iota_t[:].rearrange("p (o f) -> p o f", o=1)
                            .to_broadcast([P, nj, P]),
                        in1=dstl_t[:, p0:p0 + nj]
                            .rearrange("p (s o) -> p s o", o=1)
                            .to_broadcast([P, nj, P]),
                        op=mybir.AluOpType.is_equal)
                    ps2 = pp2.tile([P, zw], F32, tag="ps2")
                    first = True
                    for i, (t, q, a, b) in enumerate(pieces[j]):
                        gt, k = msg(gtiles2, c0 + t)
                        nc.tensor.matmul(
                            out=ps2[:],
                            lhsT=oh[:, i * P:(i + 1) * P],
                            rhs=gt[:, k * P + q * zw:k * P + (q + 1) * zw],
                            start=first, stop=False)
                        first = False
                    # self-loop: += z_own[:, j] via ident lhsT
                    nc.tensor.matmul(
                        out=ps2[:], lhsT=ident[:],
                        rhs=z_sb[:, j * zw:(j + 1) * zw],
                        start=first, stop=True)
                    nc.scalar.activation(
                        msb[:, j * zw:(j + 1) * zw], ps2[:],
                        mybir.ActivationFunctionType.Copy,
                        scale=disv_o[:, j:j + 1])

            # += bias, then split mu / logstd
            nc.vector.tensor_tensor(
                out=msb[:].rearrange("p (b c) -> p b c", c=zw),
                in0=msb[:].rearrange("p (b c) -> p b c", c=zw),
                in1=bz_t[:].rearrange("p (o c) -> p o c", o=1)
                    .to_broadcast([P, nbc, zw]),
                op=mybir.AluOpType.add)
            mv = msb[:].rearrange("p (b two c) -> p b two c", two=2, c=ow)
            nc.sync.dma_start(
                out=mu_o.ap().rearrange("pp (b c) -> pp b c", c=ow),
                in_=mv[:, :, 0])
            nc.sync.dma_start(
                out=ls_o.ap().rearrange("pp (b c) -> pp b c", c=ow),
                in_=mv[:, :, 1])

    nc.compile()
    return nc


# ---------------------------------------------------------------- entry point
def kernel(x, edge_index, W1, b1, W_mu, b_mu, W_logstd, b_logstd,
           _want_results=False, _run_kwargs=None):
    x = np.asarray(x, np.float32)
    in_ch, hid = W1.shape[0], W1.shape[1]
    ow = W_mu.shape[1]
    zw = 2 * ow
    meta, arrays = _prep(x, edge_index)

    key = (meta["N"], meta["E"], in_ch, hid, zw, meta["T"], meta["pieces"])
    if key not in _CACHE:
        _CACHE[key] = _build(meta, in_ch, hid, zw)
    nc = _CACHE[key]

    b1_b = np.tile(np.asarray(b1, np.float32), (P, 1))
    wz = np.concatenate([np.asarray(W_mu, np.float32),
                         np.asarray(W_logstd, np.float32)], axis=1)
    bz = np.concatenate([np.asarray(b_mu, np.float32),
                         np.asarray(b_logstd, np.float32)])
    bz_b = np.tile(bz, (P, 1))

    in_maps = []
    for c in range(N_CORES):
        m = dict(
            x_g=arrays["x_g"], deg_g=arrays["deg_g"],
            x_own=arrays["x_own"][c], deg_own=arrays["deg_own"][c],
            dstl=arrays["dstl"][c], iota=arrays["iota"],
            w1=np.asarray(W1, np.float32), b1=b1_b,
            wz=wz, bz=bz_b)
        for q in range(NQ):
            m[f"idxw1_{q}"] = arrays["idxw1"][q][c]
            m[f"idxw2_{q}"] = arrays["idxw2"][q][c]
        in_maps.append(m)

    res = run_bass_kernel_spmd(nc, in_maps, core_ids=list(range(N_CORES)),
                               **(_run_kwargs or {}))

    N, nbc, npc = meta["N"], meta["nbc"], meta["npc"]
    ow_ = zw // 2
    perm = meta["perm"]
    mu = np.empty((N_CORES * npc, ow_), np.float32)
    ls = np.empty((N_CORES * npc, ow_), np.float32)
    for c in range(N_CORES):
        mo = res.results[c]["mu_o"].reshape(P, nbc, ow_).transpose(1, 0, 2)
        lo = res.results[c]["ls_o"].reshape(P, nbc, ow_).transpose(1, 0, 2)
        blk = c * npc + perm[c] * P
        for j in range(nbc):
            mu[blk[j]:blk[j] + P] = mo[j]
            ls[blk[j]:blk[j] + P] = lo[j]
    out = (mu[:N], ls[:N])
    if _want_results:
        return out, res
    return out
